# revision 21
# baseline (speedup 1.0000x reference)
"""Trainium2 Bass kernel for sparse (top-k) multi-head causal attention.

Problem (hardcoded shapes, from the reference):
  B=32, S=512, D=512, H=8, DK=64, k_index=5 (any k<=8 supported)
  out = TopKCausalAttention(q, k, v; w_q..w_o, b_q..b_o)

Sharding: data-parallel over batch across 8 NeuronCores (4 batches/core).

Precision scheme (the top-k selection is discontinuous, so the q/k
projection + QK^T path needs ~17+ mantissa bits of score accuracy; plain
fp32 matmuls cost 4 cyc/row on the PE vs 1 for 16-bit):
  - hi/lo decomposition: every selection-path matmul is a single fp16
    "main" matmul (exact e10m10 products, fp32 PSUM accumulation) plus one
    fp8e4m3 DoubleRow matmul (0.5 cyc/row) that carries BOTH first-order
    correction terms (w*x_res and w_res*x) as its two K-subtiles.
  - all operands pre-scaled by powers of two so main and correction
    products land on a common PSUM scale (2^17 for projections, 2^15 for
    scores); the descale rides the exp() activation's scale input and the
    evacuation copies.  End-to-end score error ~1e-5 abs; measured
    rel_l2 vs the fp32 reference = 4.4e-3 (numpy bit-exact model).
  - value path (v projection, p transposes, pV, out projection) in fp16
    as before.

Per-core pipeline per batch b (unchanged downstream of the scores):
  scores_psum = qk mains + DR crosses (+ bf16 identity-matmul causal mask)
  e = exp(scores * 2^-15)           (ACT, PSUM->SBUF)
  top8 = vector.max(e); tau/Z/renorm; p = (e >= tau) * e * (1/Z)
  pT via PE transposes; attnT += vh^T @ pT; y = attnT^T @ w_o -> DRAM
"""

import math
import os

os.environ.setdefault("MYCRO_LOCAL_CACHE", "1")

from contextlib import ExitStack

import numpy as np
import ml_dtypes

import concourse.bass as bass
import concourse.bacc as bacc
import concourse.mybir as mybir
import concourse.tile as tile
from concourse.bass_utils import run_bass_kernel_spmd

B, S, D, H = 32, 512, 512, 8
DK = D // H  # 64
NCORES = 8
BC = B // NCORES  # batches per core
RT = S // 128  # row tiles per sequence
FT = D // 128  # feature tiles
NEG = -1.0e32

F32 = mybir.dt.float32
BF16 = mybir.dt.bfloat16
F16 = mybir.dt.float16
F8 = mybir.dt.float8e4
NPF8 = ml_dtypes.float8_e4m3

_last_nc = None

CFG = {"trace": False}

# power-of-two scale exponents (see module docstring)
#   proj psum: 2^17 = (w_hi*2^8)(x_hi*2^9); fp8 planes (w*2^6)(xres*2^11),
#              (wres*2^16)(x*2^1)
#   scores:    2^15 = (qh_hi*2^7)(kh_hi*2^8); fp8 (qres*2^13)(khi*2^2),
#              (qhi*2^2)(kres*2^13)
E_WH, E_XH = 8, 9
E_W8, E_XR8 = 6, 11
E_WR8, E_XF8 = 16, 1
E_QH, E_KH = 7, 8  # qhT_hi/khT_hi sbuf scales
SC_SCORE = 2.0 ** -15


def _f8(a):
    return np.clip(np.asarray(a, np.float32), -240, 240).astype(NPF8)


def _build_program(k_index: int):
    """Builds the per-core Bass program."""
    nc = bacc.Bacc(
        "TRN2", target_bir_lowering=False, debug=False, num_devices=NCORES
    )

    # --- DRAM I/O -------------------------------------------------------
    qTh = nc.dram_tensor("qTh", (BC, D, S), F16, kind="ExternalInput").ap()
    kTh = nc.dram_tensor("kTh", (BC, D, S), F16, kind="ExternalInput").ap()
    qc8 = nc.dram_tensor("qc8", (BC, FT, 128, 2, S), F8, kind="ExternalInput").ap()
    kc8 = nc.dram_tensor("kc8", (BC, FT, 128, 2, S), F8, kind="ExternalInput").ap()
    vT = nc.dram_tensor("vT", (BC, D, S), F16, kind="ExternalInput").ap()
    wqh = nc.dram_tensor("wqh", (D, D), F16, kind="ExternalInput").ap()
    wkh = nc.dram_tensor("wkh", (D, D), F16, kind="ExternalInput").ap()
    wqc8 = nc.dram_tensor("wqc8", (FT, 128, 2, D), F8, kind="ExternalInput").ap()
    wkc8 = nc.dram_tensor("wkc8", (FT, 128, 2, D), F8, kind="ExternalInput").ap()
    wv = nc.dram_tensor("wv", (D, D), F16, kind="ExternalInput").ap()
    wo = nc.dram_tensor("wo", (D, D), F16, kind="ExternalInput").ap()
    out = nc.dram_tensor("out", (BC, S, D), F32, kind="ExternalOutput").ap()

    # --- inline constants ----------------------------------------------
    ident_np = np.eye(128, dtype=np.float32)
    mask_np = np.where(
        np.arange(128)[None, :] >= np.arange(128)[:, None], NEG, 0.0
    ).astype(np.float32)
    ident_p = nc.inline_tensor(
        ident_np.astype(np.float16), name="identp"
    ).ap()
    ident_b = nc.inline_tensor(
        ident_np.astype(mybir.dt.np(BF16)), name="identb"
    ).ap()
    maskT_b = nc.inline_tensor(
        mask_np.T.copy().astype(mybir.dt.np(BF16)), name="maskT"
    ).ap()

    DR = mybir.MatmulPerfMode.DoubleRow
    Exp = mybir.ActivationFunctionType.Exp
    Copy = mybir.ActivationFunctionType.Copy
    AO = mybir.AluOpType

    with tile.TileContext(nc) as tc, ExitStack() as ctx:
        # ---------------- pools ----------------
        consts = ctx.enter_context(tc.tile_pool(name="consts", bufs=1))
        xpool = ctx.enter_context(tc.tile_pool(name="xpool", bufs=2))
        projpool = ctx.enter_context(tc.tile_pool(name="projpool", bufs=2))
        lopool = ctx.enter_context(tc.tile_pool(name="lopool", bufs=2))
        epool = ctx.enter_context(tc.tile_pool(name="epool", bufs=18))
        ppool = ctx.enter_context(tc.tile_pool(name="ppool", bufs=6))
        pnpool = ctx.enter_context(tc.tile_pool(name="pnpool", bufs=12))
        ptpool = ctx.enter_context(tc.tile_pool(name="ptpool", bufs=8))
        smallpool = ctx.enter_context(tc.tile_pool(name="smallpool", bufs=4))
        atpool = ctx.enter_context(tc.tile_pool(name="atpool", bufs=3))
        ypool = ctx.enter_context(tc.tile_pool(name="ypool", bufs=3))

        ps_proj = ctx.enter_context(tc.tile_pool(name="ps_proj", bufs=2, space="PSUM"))
        ps_sc = ctx.enter_context(tc.tile_pool(name="ps_sc", bufs=2, space="PSUM"))
        ps_pt = ctx.enter_context(tc.tile_pool(name="ps_pt", bufs=2, space="PSUM"))
        ps_at = ctx.enter_context(tc.tile_pool(name="ps_at", bufs=1, space="PSUM"))
        ps_y = ctx.enter_context(tc.tile_pool(name="ps_y", bufs=1, space="PSUM"))

        # ---------------- resident constants ----------------
        # q weights first, then batch 0's q activations, so the first
        # projection matmuls start as early as possible on the DMA queue.
        wqh_sb = [consts.tile_from(wqh[ft * 128:(ft + 1) * 128, :], name=f"wqh{ft}")
                  for ft in range(FT)]
        wqc8_sb = [consts.tile_from(wqc8[ft], name=f"wqc8{ft}") for ft in range(FT)]
        _xq0h = [xpool.tile_from(qTh[0, ft * 128:(ft + 1) * 128, :],
                                 name=f"xqh{ft}") for ft in range(FT)]
        _xq0c = [xpool.tile_from(qc8[0, ft], name=f"xqc{ft}") for ft in range(FT)]
        wkh_sb = [consts.tile_from(wkh[ft * 128:(ft + 1) * 128, :], name=f"wkh{ft}")
                  for ft in range(FT)]
        wkc8_sb = [consts.tile_from(wkc8[ft], name=f"wkc8{ft}") for ft in range(FT)]
        preloaded = {}
        preloaded[0] = (
            _xq0h, _xq0c,
            [xpool.tile_from(kTh[0, ft * 128:(ft + 1) * 128, :],
                             name=f"xkh{ft}") for ft in range(FT)],
            [xpool.tile_from(kc8[0, ft], name=f"xkc{ft}") for ft in range(FT)],
            [xpool.tile_from(vT[0, ft * 128:(ft + 1) * 128, :],
                             name=f"xv{ft}") for ft in range(FT)],
        )
        wv_sb = [consts.tile_from(wv[ft * 128:(ft + 1) * 128, :], name=f"wv{ft}")
                 for ft in range(FT)]
        wo_sb = [consts.tile_from(wo[dt * 128:(dt + 1) * 128, :], name=f"wo{dt}")
                 for dt in range(FT)]
        identp_sb = consts.tile_from(ident_p, name="identp_sb")
        identb_sb = consts.tile_from(ident_b, name="identb_sb")
        maskT_sb = consts.tile_from(maskT_b, name="maskT_sb")

        def emit_proj(b, defer_v=False):
            """Loads + q/k/v projections for batch b.

            q/k: per head-pair dt, one 2^17-scaled PSUM accumulates 4 fp16
            main matmuls + 4 fp8 DoubleRow correction matmuls; evacuated as
            a 2^E_QH/E_KH-scaled fp16 hi tile, an fp16 lo residual, and a
            stacked fp8 [lo|hi] (q) / [hi|lo] (k) pair tile for the QK
            cross-term DoubleRow matmuls.
            """
            if b in preloaded:
                xqh, xqc, xkh, xkc, xv = preloaded.pop(b)
            else:
                xqh = [xpool.tile_from(qTh[b, ft * 128:(ft + 1) * 128, :],
                                       name=f"xqh{ft}") for ft in range(FT)]
                xqc = [xpool.tile_from(qc8[b, ft], name=f"xqc{ft}")
                       for ft in range(FT)]
                xkh = [xpool.tile_from(kTh[b, ft * 128:(ft + 1) * 128, :],
                                       name=f"xkh{ft}") for ft in range(FT)]
                xkc = [xpool.tile_from(kc8[b, ft], name=f"xkc{ft}")
                       for ft in range(FT)]
                xv = [xpool.tile_from(vT[b, ft * 128:(ft + 1) * 128, :],
                                      name=f"xv{ft}") for ft in range(FT)]
            qhT, khT, qc8t, kc8t, vh = [], [], [], [], []
            for dt in range(FT):
                for which, wh_sb, wc_sb, xh, xc in (
                        ("q", wqh_sb, wqc8_sb, xqh, xqc),
                        ("k", wkh_sb, wkc8_sb, xkh, xkc)):
                    ps = ps_proj.tile([128, S], F32, name="psq", tag="psproj")
                    for ft in range(FT):
                        nc.tensor.matmul(
                            ps, wh_sb[ft][:, dt * 128:(dt + 1) * 128], xh[ft],
                            start=(ft == 0), stop=False)
                    for ft in range(FT):
                        nc.tensor.matmul(
                            ps, wc_sb[ft][:, :, dt * 128:(dt + 1) * 128], xc[ft],
                            start=False, stop=(ft == FT - 1), perf_mode=DR)
                    hi = projpool.tile([128, S], F16, name=f"{which}hT{dt}",
                                       tag=f"{which}hT{dt}")
                    lo = lopool.tile([128, S], F16, name=f"{which}lo",
                                     tag=f"{which}lo{dt % 2}")
                    c8 = projpool.tile([128, 2 * S], F8, name=f"{which}c8{dt}",
                                       tag=f"{which}c8{dt}")
                    eh = E_QH if which == "q" else E_KH
                    # psum 2^17 -> hi 2^eh (ACT), lo residual (DVE; GPSIMD
                    # cannot read PSUM), fp8 planes (DVE, f16->f8)
                    nc.scalar.activation(hi, ps, Copy, scale=2.0 ** (eh - 17))
                    nc.vector.scalar_tensor_tensor(
                        lo, ps, 2.0 ** (eh - 17), hi,
                        op0=AO.mult, op1=AO.subtract)
                    if which == "q":
                        # plane0 = qres*2^13 (Pool), plane1 = qhi*2^2 (DVE)
                        nc.gpsimd.tensor_scalar(
                            c8[:, 0:S], lo, 2.0 ** (13 - eh), None, op0=AO.mult)
                        nc.vector.tensor_scalar(
                            c8[:, S:2 * S], hi, 2.0 ** (2 - eh), None, op0=AO.mult)
                        qhT.append(hi)
                        qc8t.append(c8)
                    else:
                        # plane0 = khi*2^2 (DVE), plane1 = kres*2^13 (Pool)
                        nc.vector.tensor_scalar(
                            c8[:, 0:S], hi, 2.0 ** (2 - eh), None, op0=AO.mult)
                        nc.gpsimd.tensor_scalar(
                            c8[:, S:2 * S], lo, 2.0 ** (13 - eh), None, op0=AO.mult)
                        khT.append(hi)
                        kc8t.append(c8)

            def do_vproj(rts=range(RT)):
                for rt in rts:
                    ps = ps_proj.tile([128, D], F32, name="psv", tag="psproj")
                    for ft in range(FT):
                        nc.tensor.matmul(
                            ps, xv[ft][:, rt * 128:(rt + 1) * 128], wv_sb[ft],
                            start=(ft == 0), stop=(ft == FT - 1))
                    t = projpool.tile([128, D], F16, name=f"vh{rt}", tag=f"vh{rt}")
                    nc.scalar.copy(t, ps)
                    vh.append(t)
                return vh
            if defer_v:
                return qhT, khT, qc8t, kc8t, do_vproj
            return qhT, khT, qc8t, kc8t, do_vproj()

        def emit_headpair(hp, qhT, khT, qc8t, kc8t, vh):
            """Scores / top-k softmax / transposes / attnT for one head pair."""
            qc8v = qc8t[hp].rearrange("p (two s) -> p two s", two=2)
            kc8v = kc8t[hp].rearrange("p (two s) -> p two s", two=2)
            etiles = [[None] * RT, [None] * RT]
            zfulls = [None, None]
            top8s = []
            for hh in range(2):
                top8s.append(smallpool.tile(
                    [128, RT * 8], F32, name=f"top8{hh}", tag=f"top8{hh}"))
            for ri in range(RT):
                w = (ri + 1) * 128
                spss = []
                for hh in range(2):
                    po = hh * 64
                    sps = ps_sc.tile([128, S], F32, name="sps", tag="sps")
                    nc.tensor.matmul(
                        sps[:, 0:w],
                        qhT[hp][po:po + 64, ri * 128:(ri + 1) * 128],
                        khT[hp][po:po + 64, 0:w],
                        start=True, stop=False)
                    spss.append(sps)
                for hh in range(2):
                    po = hh * 64
                    nc.tensor.matmul(
                        spss[hh][:, 0:w],
                        qc8v[po:po + 64, :, ri * 128:(ri + 1) * 128],
                        kc8v[po:po + 64, :, 0:w],
                        start=False, stop=False, perf_mode=DR)
                for hh in range(2):
                    nc.tensor.matmul(
                        spss[hh][:, ri * 128:(ri + 1) * 128],
                        maskT_sb, identb_sb, start=False, stop=True)
                # exp/max8 are the latency-critical chain: they free the
                # scores PSUM bank and gate the whole downstream wave, so
                # they must preempt bulk evacuation copies in the greedy
                # per-engine schedule.
                with tc.high_priority():
                    for hh in range(2):
                        e = epool.tile([128, S], F32, name="e", tag="e")
                        if ri == 0:
                            zf = smallpool.tile(
                                [128, 1], F32, name=f"zfull{hh}", tag=f"zfull{hh}")
                            zfulls[hh] = zf
                            nc.scalar.activation(
                                e[:, 0:w], spss[hh][:, 0:w], Exp, scale=SC_SCORE,
                                accum_out=zf)
                        else:
                            nc.scalar.activation(
                                e[:, 0:w], spss[hh][:, 0:w], Exp, scale=SC_SCORE)
                        nc.vector.max(
                            out=top8s[hh][:, ri * 8:(ri + 1) * 8], in_=e[:, 0:w])
                        etiles[hh][ri] = e
            ptrows = [[None] * RT, [None] * RT]
            for hh in range(2):
                top8 = top8s[hh]
                zk = smallpool.tile([128, RT], F32, name="zk", tag="zk")
                with tc.high_priority():
                    nc.vector.reduce_sum(
                        zk,
                        top8.rearrange("p (r e) -> p r e", e=8)[:, :, 0:k_index],
                        axis=mybir.AxisListType.X)
                    nc.vector.tensor_copy(
                        zk[0:k_index, 0:1], zfulls[hh][0:k_index, :])
                    nc.vector.memset(zk[0:1, 0:1], 1.0)
                    nc.vector.memset(top8[0:k_index, k_index - 1:k_index], 0.0)
                    rz = smallpool.tile([128, RT], F32, name="rz", tag="rz")
                    nc.vector.reciprocal(rz, zk)

                pns = []
                for ri in range(RT):
                    w = (ri + 1) * 128
                    e = etiles[hh][ri]
                    tau = top8[:, ri * 8 + k_index - 1: ri * 8 + k_index]
                    pu = ppool.tile([128, S], F16, name="pu", tag="pu")
                    nc.vector.scalar_tensor_tensor(
                        pu[:, 0:w], e[:, 0:w], tau, e[:, 0:w],
                        op0=AO.is_ge, op1=AO.mult)
                    pn = pnpool.tile([128, S], F16, name="pn", tag="pn")
                    nc.gpsimd.tensor_scalar(
                        pn[:, 0:w], pu[:, 0:w], rz[:, ri:ri + 1], None,
                        op0=AO.mult)
                    pns.append(pn)
                # transposes for two column-tiles share one full PSUM bank
                # and get evacuated by a single wide copy
                for pp in range(2):
                    cis = [2 * pp, 2 * pp + 1]
                    ptb = ps_pt.tile([128, 2 * S], F16, name="ptb", tag="ptb")
                    off = 0
                    offs = []
                    for c in cis:
                        for ri in range(c, RT):
                            nc.tensor.transpose(
                                ptb[:, off + (ri - c) * 128:
                                    off + (ri - c + 1) * 128],
                                pns[ri][:, c * 128:(c + 1) * 128],
                                identp_sb)
                        offs.append(off)
                        off += (RT - c) * 128
                    ptrow = ptpool.tile([128, 2 * S], F16, name="ptrow",
                                        tag="ptrow")
                    if pp == 0:
                        nc.vector.tensor_copy(ptrow[:, 0:off], ptb[:, 0:off])
                    else:
                        nc.scalar.copy(ptrow[:, 0:off], ptb[:, 0:off])
                    for c, o in zip(cis, offs):
                        ptrows[hh][c] = ptrow[:, o:o + (RT - c) * 128]

            def finish(vh):
                at_ps = ps_at.tile([128, S], F32, name="atps", tag="atps")
                for ci in range(RT):
                    wv_ = (RT - ci) * 128
                    for hh in range(2):
                        h = 2 * hp + hh
                        nc.tensor.matmul(
                            at_ps[hh * 64:hh * 64 + 64, ci * 128:S],
                            vh[ci][:, h * DK:(h + 1) * DK],
                            ptrows[hh][ci][:, 0:wv_],
                            start=(ci == 0), stop=(ci == RT - 1),
                            skip_group_check=True)
                at = atpool.tile([128, S], F16, name=f"at{hp}", tag=f"at{hp}")
                nc.scalar.copy(at, at_ps)
                return at
            if vh is None:
                return finish
            return finish(vh)

        def emit_y(b, attnT_sb):
            for ri in range(RT):
                yps = ps_y.tile([128, D], F32, name="yps", tag="yps")
                for hp in range(FT):
                    nc.tensor.matmul(
                        yps, attnT_sb[hp][:, ri * 128:(ri + 1) * 128], wo_sb[hp],
                        start=(hp == 0), stop=(hp == FT - 1))
                y = ypool.tile([128, D], F32, name="y", tag="y")
                nc.scalar.copy(y, yps)
                nc.scalar.dma_start(out[b, ri * 128:(ri + 1) * 128, :], y)

        for b in range(BC):
            last = b == BC - 1
            qhT, khT, qc8t, kc8t, vh = emit_proj(b, defer_v=last)
            attnT_sb = []
            if last:
                # cooldown filler: spread the last batch's v-projection
                # groups across the head-pair phases
                do_v = vh
                fins = []
                vh = None
                for hp in range(FT):
                    fins.append(emit_headpair(hp, qhT, khT, qc8t, kc8t, None))
                    vh = do_v(rts=[hp])
                attnT_sb = [fin(vh) for fin in fins]
            else:
                for hp in range(FT):
                    attnT_sb.append(
                        emit_headpair(hp, qhT, khT, qc8t, kc8t, vh))
            emit_y(b, attnT_sb)

    nc.compile()
    return nc


def _prep_side(x, w):
    """Host split of one projection input pair.

    x: [n, S, D] fp32 activations, w: [D, D] fp32 weights (score scale
    pre-folded for q).  Returns (xTh fp16 [n,D,S], xc8 fp8 [n,FT,128,2,S],
    wh fp16 [D,D], wc8 fp8 [FT,128,2,D]).
    """
    x = np.asarray(x, np.float64)
    w = np.asarray(w, np.float64)
    x_hi = x.astype(np.float32).astype(np.float16)
    x_res = x - x_hi.astype(np.float64)
    w_hi = w.astype(np.float32).astype(np.float16)
    w_res = w - w_hi.astype(np.float64)

    xTh = np.ascontiguousarray(
        (x_hi.astype(np.float32) * 2.0 ** E_XH).astype(np.float16)
        .transpose(0, 2, 1))
    n = x.shape[0]
    xc8 = np.empty((n, FT, 128, 2, S), NPF8)
    p0 = _f8(x_res * 2.0 ** E_XR8).transpose(0, 2, 1).reshape(n, FT, 128, S)
    p1 = _f8(x * 2.0 ** E_XF8).transpose(0, 2, 1).reshape(n, FT, 128, S)
    xc8[:, :, :, 0, :] = p0
    xc8[:, :, :, 1, :] = p1

    wh = np.ascontiguousarray(
        (w_hi.astype(np.float32) * 2.0 ** E_WH).astype(np.float16))
    wc8 = np.empty((FT, 128, 2, D), NPF8)
    wc8[:, :, 0, :] = _f8(w * 2.0 ** E_W8).reshape(FT, 128, D)
    wc8[:, :, 1, :] = _f8(w_res * 2.0 ** E_WR8).reshape(FT, 128, D)
    return xTh, np.ascontiguousarray(xc8), wh, np.ascontiguousarray(wc8)


def kernel(**inputs):
    q = np.asarray(inputs["q"], np.float32)
    k = np.asarray(inputs["k"], np.float32)
    v = np.asarray(inputs["v"], np.float32)
    w_q = np.asarray(inputs["w_q"], np.float32)
    w_k = np.asarray(inputs["w_k"], np.float32)
    w_v = np.asarray(inputs["w_v"], np.float32)
    w_o = np.asarray(inputs["w_o"], np.float32)
    b_q = np.asarray(inputs["b_q"], np.float32)
    b_k = np.asarray(inputs["b_k"], np.float32)
    b_v = np.asarray(inputs["b_v"], np.float32)
    b_o = np.asarray(inputs["b_o"], np.float32)
    k_index = int(np.asarray(inputs["k_index"]))
    assert 1 <= k_index <= 8, f"kernel supports k_index<=8, got {k_index}"
    assert not (np.any(b_q) or np.any(b_k) or np.any(b_v) or np.any(b_o)), (
        "this kernel build assumes zero biases")

    # fold the 1/sqrt(DK) score scaling into the q projection
    w_qs = (w_q.astype(np.float64) / math.sqrt(DK))

    nc = _build_program(k_index)
    global _last_nc
    _last_nc = nc

    _, _, wqh, wqc8 = _prep_side(q[:1], w_qs)
    _, _, wkh, wkc8 = _prep_side(k[:1], w_k)
    shared = {
        "wqh": wqh, "wqc8": wqc8, "wkh": wkh, "wkc8": wkc8,
        "wv": np.ascontiguousarray(w_v.astype(np.float16)),
        "wo": np.ascontiguousarray(w_o.astype(np.float16)),
    }

    in_maps = []
    for c in range(NCORES):
        sl = slice(c * BC, (c + 1) * BC)
        qTh, qc8_, _, _ = _prep_side(q[sl], w_qs)
        kTh, kc8_, _, _ = _prep_side(k[sl], w_k)
        in_maps.append(dict(
            shared,
            qTh=qTh, qc8=qc8_, kTh=kTh, kc8=kc8_,
            vT=np.ascontiguousarray(
                v[sl].transpose(0, 2, 1).astype(np.float16)),
        ))

    res = run_bass_kernel_spmd(
        nc, in_maps, core_ids=list(range(NCORES)), trace=CFG["trace"]
    )
    out = np.concatenate([r["out"] for r in res.results], axis=0)
    kernel.last_result = res
    return out


# revision 44
# speedup vs baseline: 1.0691x; 1.0691x over previous
"""Trainium2 Bass kernel for sparse (top-k) multi-head causal attention.

Problem (hardcoded shapes, from the reference):
  B=32, S=512, D=512, H=8, DK=64, k_index=5 (any k<=8 supported)
  out = TopKCausalAttention(q, k, v; w_q..w_o, b_q..b_o)

Sharding: data-parallel over batch across 8 NeuronCores (4 batches/core).

Precision scheme (the top-k selection is discontinuous, so the q/k
projection + QK^T path needs ~17+ mantissa bits of score accuracy; plain
fp32 matmuls cost 4 cyc/row on the PE vs 1 for 16-bit):
  - hi/lo decomposition: every selection-path matmul is a single fp16
    "main" matmul (exact e10m10 products, fp32 PSUM accumulation) plus one
    fp8e4m3 DoubleRow matmul (0.5 cyc/row) that carries BOTH first-order
    correction terms (w*x_res and w_res*x) as its two K-subtiles.
  - all operands pre-scaled by powers of two so main and correction
    products land on a common PSUM scale (2^17 for projections, 2^15 for
    scores); the descale rides the exp() activation's scale input and the
    evacuation copies.  End-to-end score error ~1e-5 abs; measured
    rel_l2 vs the fp32 reference = 4.4e-3 (numpy bit-exact model).
  - value path (v projection, p transposes, pV, out projection) in fp16
    as before.

Per-core pipeline per batch b (unchanged downstream of the scores):
  scores_psum = qk mains + DR crosses (+ bf16 identity-matmul causal mask)
  e = exp(scores * 2^-15)           (ACT, PSUM->SBUF)
  top8 = vector.max(e); tau/Z/renorm; p = (e >= tau) * e * (1/Z)
  pT via PE transposes; attnT += vh^T @ pT; y = attnT^T @ w_o -> DRAM
"""

import math
import os

os.environ.setdefault("MYCRO_LOCAL_CACHE", "1")

from contextlib import ExitStack

import numpy as np
import ml_dtypes

import concourse.bass as bass
import concourse.bacc as bacc
import concourse.mybir as mybir
import concourse.tile as tile
from concourse.bass_utils import run_bass_kernel_spmd

B, S, D, H = 32, 512, 512, 8
DK = D // H  # 64
NCORES = 8
BC = B // NCORES  # batches per core
RT = S // 128  # row tiles per sequence
FT = D // 128  # feature tiles
NEG = -1.0e32

F32 = mybir.dt.float32
BF16 = mybir.dt.bfloat16
F16 = mybir.dt.float16
F8 = mybir.dt.float8e4
NPF8 = ml_dtypes.float8_e4m3

_last_nc = None

CFG = {
    "trace": False,
    # pool buffer counts (PSUM total must fit 8 banks)
    "ps_proj": 2, "ps_sc": 2, "ps_pt": 2, "ps_at": 1, "ps_y": 1,
    "epool": 18, "ppool": 6, "pnpool": 12, "ptpool": 8, "xpool": 2,
    "projpool": 2,
}

# power-of-two scale exponents (see module docstring)
#   proj psum: 2^17 = (w_hi*2^8)(x_hi*2^9); fp8 planes (w*2^6)(xres*2^11),
#              (wres*2^16)(x*2^1)
#   scores:    2^15 = (qh_hi*2^7)(kh_hi*2^8); fp8 (qres*2^13)(khi*2^2),
#              (qhi*2^2)(kres*2^13)
E_WH, E_XH = 8, 9
E_W8, E_XR8 = 6, 11
E_WR8, E_XF8 = 16, 1
E_QH, E_KH = 7, 8  # qhT_hi/khT_hi sbuf scales
SC_SCORE = 2.0 ** -15


def _f8(a):
    return np.clip(np.asarray(a, np.float32), -240, 240).astype(NPF8)


def _build_program(k_index: int):
    """Builds the per-core Bass program."""
    nc = bacc.Bacc(
        "TRN2", target_bir_lowering=False, debug=False, num_devices=NCORES
    )

    # --- DRAM I/O -------------------------------------------------------
    qTh = nc.dram_tensor("qTh", (BC, D, S), F16, kind="ExternalInput").ap()
    kTh = nc.dram_tensor("kTh", (BC, D, S), F16, kind="ExternalInput").ap()
    qc8 = nc.dram_tensor("qc8", (BC, FT, 128, 2, S), F8, kind="ExternalInput").ap()
    kc8 = nc.dram_tensor("kc8", (BC, FT, 128, 2, S), F8, kind="ExternalInput").ap()
    vT = nc.dram_tensor("vT", (BC, D, S), F16, kind="ExternalInput").ap()
    wqh = nc.dram_tensor("wqh", (D, D), F16, kind="ExternalInput").ap()
    wkh = nc.dram_tensor("wkh", (D, D), F16, kind="ExternalInput").ap()
    wqc8 = nc.dram_tensor("wqc8", (FT, 128, 2, D), F8, kind="ExternalInput").ap()
    wkc8 = nc.dram_tensor("wkc8", (FT, 128, 2, D), F8, kind="ExternalInput").ap()
    wv = nc.dram_tensor("wv", (D, D), F16, kind="ExternalInput").ap()
    wo = nc.dram_tensor("wo", (D, D), F16, kind="ExternalInput").ap()
    out = nc.dram_tensor("out", (BC, S, D), F32, kind="ExternalOutput").ap()

    # --- inline constants ----------------------------------------------
    ident_np = np.eye(128, dtype=np.float32)
    mask_np = np.where(
        np.arange(128)[None, :] >= np.arange(128)[:, None], NEG, 0.0
    ).astype(np.float32)
    ident_p = nc.inline_tensor(
        ident_np.astype(np.float16), name="identp"
    ).ap()
    ident_b = nc.inline_tensor(
        ident_np.astype(mybir.dt.np(BF16)), name="identb"
    ).ap()
    maskT_b = nc.inline_tensor(
        mask_np.T.copy().astype(mybir.dt.np(BF16)), name="maskT"
    ).ap()

    DR = mybir.MatmulPerfMode.DoubleRow
    Exp = mybir.ActivationFunctionType.Exp
    Copy = mybir.ActivationFunctionType.Copy
    AO = mybir.AluOpType

    with tile.TileContext(nc) as tc, ExitStack() as ctx:
        # ---------------- pools ----------------
        consts = ctx.enter_context(tc.tile_pool(name="consts", bufs=1))
        xpool = ctx.enter_context(tc.tile_pool(name="xpool", bufs=CFG["xpool"]))
        projpool = ctx.enter_context(
            tc.tile_pool(name="projpool", bufs=CFG["projpool"]))
        lopool = ctx.enter_context(tc.tile_pool(name="lopool", bufs=2))
        epool = ctx.enter_context(tc.tile_pool(name="epool", bufs=CFG["epool"]))
        ppool = ctx.enter_context(tc.tile_pool(name="ppool", bufs=CFG["ppool"]))
        pnpool = ctx.enter_context(tc.tile_pool(name="pnpool", bufs=CFG["pnpool"]))
        ptpool = ctx.enter_context(tc.tile_pool(name="ptpool", bufs=CFG["ptpool"]))
        smallpool = ctx.enter_context(tc.tile_pool(name="smallpool", bufs=4))
        atpool = ctx.enter_context(tc.tile_pool(name="atpool", bufs=3))
        ypool = ctx.enter_context(tc.tile_pool(name="ypool", bufs=3))

        ps_proj = ctx.enter_context(
            tc.tile_pool(name="ps_proj", bufs=CFG["ps_proj"], space="PSUM"))
        ps_sc = ctx.enter_context(
            tc.tile_pool(name="ps_sc", bufs=CFG["ps_sc"], space="PSUM"))
        ps_pt = ctx.enter_context(
            tc.tile_pool(name="ps_pt", bufs=CFG["ps_pt"], space="PSUM"))
        ps_at = ctx.enter_context(
            tc.tile_pool(name="ps_at", bufs=CFG["ps_at"], space="PSUM"))
        ps_y = ctx.enter_context(
            tc.tile_pool(name="ps_y", bufs=CFG["ps_y"], space="PSUM"))

        # ---------------- resident constants ----------------
        # One wide DMA per tensor (the SP sequencer serializes DMA issue at
        # ~565ns each, so fewer/bigger transfers matter): dram [(f p) s]
        # lands as a [128, f*s] tile whose column block f holds partition
        # rows f*128..f*128+127.
        def load_blocked(pool, ap2d, name):
            t = pool.tile_from(
                ap2d.rearrange("(f p) s -> p f s", p=128), name=name)
            return [t[:, ft] for ft in range(FT)]

        def load_c8(pool, ap4d, name):
            # dram [f p two s] -> tile [128, f, 2, s]; per-ft [128, 2, s] views
            t = pool.tile_from(
                ap4d.rearrange("f p two s -> p f two s"), name=name)
            return [t[:, ft] for ft in range(FT)]

        # q weights first, then batch 0's q activations, so the first
        # projection matmuls start as early as possible on the DMA queue.
        wqh_sb = load_blocked(consts, wqh, "wqh")
        wqc8_sb = load_c8(consts, wqc8, "wqc8")
        _xq0h = load_blocked(xpool, qTh[0], "xqh")
        _xq0c = load_c8(xpool, qc8[0], "xqc")
        wkh_sb = load_blocked(consts, wkh, "wkh")
        wkc8_sb = load_c8(consts, wkc8, "wkc8")
        preloaded = {}
        preloaded[0] = (
            _xq0h, _xq0c,
            load_blocked(xpool, kTh[0], "xkh"),
            load_c8(xpool, kc8[0], "xkc"),
            load_blocked(xpool, vT[0], "xv"),
        )
        wv_sb = load_blocked(consts, wv, "wv")
        wo_sb = load_blocked(consts, wo, "wo")
        identp_sb = consts.tile_from(ident_p, name="identp_sb")
        identb_sb = consts.tile_from(ident_b, name="identb_sb")
        maskT_sb = consts.tile_from(maskT_b, name="maskT_sb")

        def emit_proj(b, defer_v=False):
            """Loads + q/k/v projections for batch b.

            q/k: per head-pair dt, one 2^17-scaled PSUM accumulates 4 fp16
            main matmuls + 4 fp8 DoubleRow correction matmuls; evacuated as
            a 2^E_QH/E_KH-scaled fp16 hi tile, an fp16 lo residual, and a
            stacked fp8 [lo|hi] (q) / [hi|lo] (k) pair tile for the QK
            cross-term DoubleRow matmuls.
            """
            if b in preloaded:
                xqh, xqc, xkh, xkc, xv = preloaded.pop(b)
            else:
                xqh = load_blocked(xpool, qTh[b], "xqh")
                xqc = load_c8(xpool, qc8[b], "xqc")
                xkh = load_blocked(xpool, kTh[b], "xkh")
                xkc = load_c8(xpool, kc8[b], "xkc")
                xv = load_blocked(xpool, vT[b], "xv")
            qhT, khT, qc8t, kc8t, vh = [], [], [], [], []
            for dt in range(FT):
                for which, wh_sb, wc_sb, xh, xc in (
                        ("q", wqh_sb, wqc8_sb, xqh, xqc),
                        ("k", wkh_sb, wkc8_sb, xkh, xkc)):
                    ps = ps_proj.tile([128, S], F32, name="psq", tag="psproj")
                    for ft in range(FT):
                        nc.tensor.matmul(
                            ps, wh_sb[ft][:, dt * 128:(dt + 1) * 128], xh[ft],
                            start=(ft == 0), stop=False)
                    for ft in range(FT):
                        nc.tensor.matmul(
                            ps, wc_sb[ft][:, :, dt * 128:(dt + 1) * 128], xc[ft],
                            start=False, stop=(ft == FT - 1), perf_mode=DR)
                    hi = projpool.tile([128, S], F16, name=f"{which}hT{dt}",
                                       tag=f"{which}hT{dt}")
                    lo = lopool.tile([128, S], F16, name=f"{which}lo",
                                     tag=f"{which}lo{dt % 2}")
                    c8 = projpool.tile([128, 2 * S], F8, name=f"{which}c8{dt}",
                                       tag=f"{which}c8{dt}")
                    eh = E_QH if which == "q" else E_KH
                    # psum 2^17 -> hi 2^eh (ACT), lo residual (DVE; GPSIMD
                    # cannot read PSUM), fp8 planes (DVE, f16->f8)
                    nc.scalar.activation(hi, ps, Copy, scale=2.0 ** (eh - 17))
                    nc.vector.scalar_tensor_tensor(
                        lo, ps, 2.0 ** (eh - 17), hi,
                        op0=AO.mult, op1=AO.subtract)
                    if which == "q":
                        # plane0 = qres*2^13 (Pool), plane1 = qhi*2^2 (DVE)
                        nc.gpsimd.tensor_scalar(
                            c8[:, 0:S], lo, 2.0 ** (13 - eh), None, op0=AO.mult)
                        nc.vector.tensor_scalar(
                            c8[:, S:2 * S], hi, 2.0 ** (2 - eh), None, op0=AO.mult)
                        qhT.append(hi)
                        qc8t.append(c8)
                    else:
                        # plane0 = khi*2^2 (DVE), plane1 = kres*2^13 (Pool)
                        nc.vector.tensor_scalar(
                            c8[:, 0:S], hi, 2.0 ** (2 - eh), None, op0=AO.mult)
                        nc.gpsimd.tensor_scalar(
                            c8[:, S:2 * S], lo, 2.0 ** (13 - eh), None, op0=AO.mult)
                        khT.append(hi)
                        kc8t.append(c8)

            def do_vproj(rts=range(RT)):
                for rt in rts:
                    ps = ps_proj.tile([128, D], F32, name="psv", tag="psproj")
                    for ft in range(FT):
                        nc.tensor.matmul(
                            ps, xv[ft][:, rt * 128:(rt + 1) * 128], wv_sb[ft],
                            start=(ft == 0), stop=(ft == FT - 1))
                    t = projpool.tile([128, D], F16, name=f"vh{rt}", tag=f"vh{rt}")
                    nc.scalar.copy(t, ps)
                    vh.append(t)
                return vh
            if defer_v:
                return qhT, khT, qc8t, kc8t, do_vproj
            return qhT, khT, qc8t, kc8t, do_vproj()

        def emit_headpair(hp, qhT, khT, qc8t, kc8t, vh):
            """Scores / top-k softmax / transposes / attnT for one head pair."""
            qc8v = qc8t[hp].rearrange("p (two s) -> p two s", two=2)
            kc8v = kc8t[hp].rearrange("p (two s) -> p two s", two=2)
            etiles = [[None] * RT, [None] * RT]
            top8s = []
            for hh in range(2):
                top8s.append(smallpool.tile(
                    [128, RT * 8], F32, name=f"top8{hh}", tag=f"top8{hh}"))
            for ri in range(RT):
                w = (ri + 1) * 128
                spss = []
                for hh in range(2):
                    po = hh * 64
                    sps = ps_sc.tile([128, S], F32, name="sps", tag="sps")
                    nc.tensor.matmul(
                        sps[:, 0:w],
                        qhT[hp][po:po + 64, ri * 128:(ri + 1) * 128],
                        khT[hp][po:po + 64, 0:w],
                        start=True, stop=False)
                    spss.append(sps)
                for hh in range(2):
                    po = hh * 64
                    nc.tensor.matmul(
                        spss[hh][:, 0:w],
                        qc8v[po:po + 64, :, ri * 128:(ri + 1) * 128],
                        kc8v[po:po + 64, :, 0:w],
                        start=False, stop=False, perf_mode=DR)
                for hh in range(2):
                    nc.tensor.matmul(
                        spss[hh][:, ri * 128:(ri + 1) * 128],
                        maskT_sb, identb_sb, start=False, stop=True)
                for hh in range(2):
                    e = epool.tile([128, S], F32, name="e", tag="e")
                    nc.scalar.activation(
                        e[:, 0:w], spss[hh][:, 0:w], Exp, scale=SC_SCORE)
                    nc.vector.max(
                        out=top8s[hh][:, ri * 8:(ri + 1) * 8], in_=e[:, 0:w])
                    etiles[hh][ri] = e
            ptrows = [[None] * RT, [None] * RT]
            pns2 = [[], []]
            for hh in range(2):
                # Rows with fewer than k_index valid (strictly-causal) entries
                # naturally have top8[k-1] == 0, so tau == 0 keeps everything
                # and sum(top-k) equals the full row sum — no special-casing
                # needed beyond row 0 (all-zero row: Z := 1 to avoid 1/0).
                top8 = top8s[hh]
                zk = smallpool.tile([128, RT], F32, name="zk", tag="zk")
                nc.vector.reduce_sum(
                    zk,
                    top8.rearrange("p (r e) -> p r e", e=8)[:, :, 0:k_index],
                    axis=mybir.AxisListType.X)
                nc.vector.memset(zk[0:1, 0:1], 1.0)
                rz = smallpool.tile([128, RT], F32, name="rz", tag="rz")
                nc.vector.reciprocal(rz, zk)

                for ri in range(RT):
                    w = (ri + 1) * 128
                    e = etiles[hh][ri]
                    tau = top8[:, ri * 8 + k_index - 1: ri * 8 + k_index]
                    pu = ppool.tile([128, S], F16, name="pu", tag="pu")
                    nc.vector.scalar_tensor_tensor(
                        pu[:, 0:w], e[:, 0:w], tau, e[:, 0:w],
                        op0=AO.is_ge, op1=AO.mult)
                    pn = pnpool.tile([128, S], F16, name="pn", tag="pn")
                    nc.gpsimd.tensor_scalar(
                        pn[:, 0:w], pu[:, 0:w], rz[:, ri:ri + 1], None,
                        op0=AO.mult)
                    pns2[hh].append(pn)
            for hh in range(2):
                for ci in range(RT):
                    wv_ = (RT - ci) * 128
                    ptb = ps_pt.tile([128, S], F16, name="ptb", tag="ptb")
                    for ri in range(ci, RT):
                        nc.tensor.transpose(
                            ptb[:, (ri - ci) * 128:(ri - ci + 1) * 128],
                            pns2[hh][ri][:, ci * 128:(ci + 1) * 128],
                            identp_sb)
                    ptrow = ptpool.tile([128, S], F16, name="ptrow",
                                        tag="ptrow")
                    if ci % 2 == 0:
                        nc.vector.tensor_copy(ptrow[:, 0:wv_], ptb[:, 0:wv_])
                    else:
                        nc.scalar.copy(ptrow[:, 0:wv_], ptb[:, 0:wv_])
                    ptrows[hh][ci] = ptrow[:, 0:wv_]

            def finish(vh):
                at_ps = ps_at.tile([128, S], F32, name="atps", tag="atps")
                for ci in range(RT):
                    wv_ = (RT - ci) * 128
                    for hh in range(2):
                        h = 2 * hp + hh
                        nc.tensor.matmul(
                            at_ps[hh * 64:hh * 64 + 64, ci * 128:S],
                            vh[ci][:, h * DK:(h + 1) * DK],
                            ptrows[hh][ci][:, 0:wv_],
                            start=(ci == 0), stop=(ci == RT - 1),
                            skip_group_check=True)
                at = atpool.tile([128, S], F16, name=f"at{hp}", tag=f"at{hp}")
                nc.scalar.copy(at, at_ps)
                return at
            if vh is None:
                return finish
            return finish(vh)

        def emit_y(b, attnT_sb):
            for ri in range(RT):
                yps = ps_y.tile([128, D], F32, name="yps", tag="yps")
                for hp in range(FT):
                    nc.tensor.matmul(
                        yps, attnT_sb[hp][:, ri * 128:(ri + 1) * 128], wo_sb[hp],
                        start=(hp == 0), stop=(hp == FT - 1))
                y = ypool.tile([128, D], F32, name="y", tag="y")
                nc.scalar.copy(y, yps)
                nc.scalar.dma_start(out[b, ri * 128:(ri + 1) * 128, :], y)

        for b in range(BC):
            last = b == BC - 1
            qhT, khT, qc8t, kc8t, vh = emit_proj(b, defer_v=last)
            attnT_sb = []
            if last:
                # cooldown filler: spread the last batch's v-projection
                # groups across the head-pair phases
                do_v = vh
                fins = []
                vh = None
                for hp in range(FT):
                    fins.append(emit_headpair(hp, qhT, khT, qc8t, kc8t, None))
                    vh = do_v(rts=[hp])
                attnT_sb = [fin(vh) for fin in fins]
            else:
                for hp in range(FT):
                    attnT_sb.append(
                        emit_headpair(hp, qhT, khT, qc8t, kc8t, vh))
            emit_y(b, attnT_sb)

    nc.compile()
    return nc


def _prep_side(x, w):
    """Host split of one projection input pair.

    x: [n, S, D] fp32 activations, w: [D, D] fp32 weights (score scale
    pre-folded for q).  Returns (xTh fp16 [n,D,S], xc8 fp8 [n,FT,128,2,S],
    wh fp16 [D,D], wc8 fp8 [FT,128,2,D]).
    """
    x = np.asarray(x, np.float64)
    w = np.asarray(w, np.float64)
    x_hi = x.astype(np.float32).astype(np.float16)
    x_res = x - x_hi.astype(np.float64)
    w_hi = w.astype(np.float32).astype(np.float16)
    w_res = w - w_hi.astype(np.float64)

    xTh = np.ascontiguousarray(
        (x_hi.astype(np.float32) * 2.0 ** E_XH).astype(np.float16)
        .transpose(0, 2, 1))
    n = x.shape[0]
    xc8 = np.empty((n, FT, 128, 2, S), NPF8)
    p0 = _f8(x_res * 2.0 ** E_XR8).transpose(0, 2, 1).reshape(n, FT, 128, S)
    p1 = _f8(x * 2.0 ** E_XF8).transpose(0, 2, 1).reshape(n, FT, 128, S)
    xc8[:, :, :, 0, :] = p0
    xc8[:, :, :, 1, :] = p1

    wh = np.ascontiguousarray(
        (w_hi.astype(np.float32) * 2.0 ** E_WH).astype(np.float16))
    wc8 = np.empty((FT, 128, 2, D), NPF8)
    wc8[:, :, 0, :] = _f8(w * 2.0 ** E_W8).reshape(FT, 128, D)
    wc8[:, :, 1, :] = _f8(w_res * 2.0 ** E_WR8).reshape(FT, 128, D)
    return xTh, np.ascontiguousarray(xc8), wh, np.ascontiguousarray(wc8)


def kernel(**inputs):
    q = np.asarray(inputs["q"], np.float32)
    k = np.asarray(inputs["k"], np.float32)
    v = np.asarray(inputs["v"], np.float32)
    w_q = np.asarray(inputs["w_q"], np.float32)
    w_k = np.asarray(inputs["w_k"], np.float32)
    w_v = np.asarray(inputs["w_v"], np.float32)
    w_o = np.asarray(inputs["w_o"], np.float32)
    b_q = np.asarray(inputs["b_q"], np.float32)
    b_k = np.asarray(inputs["b_k"], np.float32)
    b_v = np.asarray(inputs["b_v"], np.float32)
    b_o = np.asarray(inputs["b_o"], np.float32)
    k_index = int(np.asarray(inputs["k_index"]))
    assert 1 <= k_index <= 8, f"kernel supports k_index<=8, got {k_index}"
    assert not (np.any(b_q) or np.any(b_k) or np.any(b_v) or np.any(b_o)), (
        "this kernel build assumes zero biases")

    # fold the 1/sqrt(DK) score scaling into the q projection
    w_qs = (w_q.astype(np.float64) / math.sqrt(DK))

    nc = _build_program(k_index)
    global _last_nc
    _last_nc = nc

    _, _, wqh, wqc8 = _prep_side(q[:1], w_qs)
    _, _, wkh, wkc8 = _prep_side(k[:1], w_k)
    shared = {
        "wqh": wqh, "wqc8": wqc8, "wkh": wkh, "wkc8": wkc8,
        "wv": np.ascontiguousarray(w_v.astype(np.float16)),
        "wo": np.ascontiguousarray(w_o.astype(np.float16)),
    }

    in_maps = []
    for c in range(NCORES):
        sl = slice(c * BC, (c + 1) * BC)
        qTh, qc8_, _, _ = _prep_side(q[sl], w_qs)
        kTh, kc8_, _, _ = _prep_side(k[sl], w_k)
        in_maps.append(dict(
            shared,
            qTh=qTh, qc8=qc8_, kTh=kTh, kc8=kc8_,
            vT=np.ascontiguousarray(
                v[sl].transpose(0, 2, 1).astype(np.float16)),
        ))

    res = run_bass_kernel_spmd(
        nc, in_maps, core_ids=list(range(NCORES)), trace=CFG["trace"]
    )
    out = np.concatenate([r["out"] for r in res.results], axis=0)
    kernel.last_result = res
    return out


# revision 49
# speedup vs baseline: 1.0719x; 1.0026x over previous
"""Trainium2 Bass kernel for sparse (top-k) multi-head causal attention.

Problem (hardcoded shapes, from the reference):
  B=32, S=512, D=512, H=8, DK=64, k_index=5 (any k<=8 supported)
  out = TopKCausalAttention(q, k, v; w_q..w_o, b_q..b_o)

Sharding: data-parallel over batch across 8 NeuronCores (4 batches/core).

Precision scheme (the top-k selection is discontinuous, so the q/k
projection + QK^T path needs ~17+ mantissa bits of score accuracy; plain
fp32 matmuls cost 4 cyc/row on the PE vs 1 for 16-bit):
  - hi/lo decomposition: every selection-path matmul is a single fp16
    "main" matmul (exact e10m10 products, fp32 PSUM accumulation) plus one
    fp8e4m3 DoubleRow matmul (0.5 cyc/row) that carries BOTH first-order
    correction terms (w*x_res and w_res*x) as its two K-subtiles.
  - all operands pre-scaled by powers of two so main and correction
    products land on a common PSUM scale (2^17 for projections, 2^15 for
    scores); the descale rides the exp() activation's scale input and the
    evacuation copies.  End-to-end score error ~1e-5 abs; measured
    rel_l2 vs the fp32 reference = 4.4e-3 (numpy bit-exact model).
  - value path (v projection, p transposes, pV, out projection) in fp16
    as before.

Per-core pipeline per batch b (unchanged downstream of the scores):
  scores_psum = qk mains + DR crosses (+ bf16 identity-matmul causal mask)
  e = exp(scores * 2^-15)           (ACT, PSUM->SBUF)
  top8 = vector.max(e); tau/Z/renorm; p = (e >= tau) * e * (1/Z)
  pT via PE transposes; attnT += vh^T @ pT; y = attnT^T @ w_o -> DRAM
"""

import math
import os

os.environ.setdefault("MYCRO_LOCAL_CACHE", "1")

from contextlib import ExitStack

import numpy as np
import ml_dtypes

import concourse.bass as bass
import concourse.bacc as bacc
import concourse.mybir as mybir
import concourse.tile as tile
from concourse.bass_utils import run_bass_kernel_spmd

B, S, D, H = 32, 512, 512, 8
DK = D // H  # 64
NCORES = 8
BC = B // NCORES  # batches per core
RT = S // 128  # row tiles per sequence
FT = D // 128  # feature tiles
NEG = -1.0e32

F32 = mybir.dt.float32
BF16 = mybir.dt.bfloat16
F16 = mybir.dt.float16
F8 = mybir.dt.float8e4
NPF8 = ml_dtypes.float8_e4m3

_last_nc = None

CFG = {
    "trace": False,
    # pool buffer counts (PSUM total must fit 8 banks)
    "ps_proj": 2, "ps_sc": 2, "ps_pt": 2, "ps_at": 1, "ps_y": 1,
    "epool": 18, "ppool": 6, "pnpool": 12, "ptpool": 8, "xpool": 2,
    "projpool": 2,
    # engine placement toggles
    "pn_eng": "pool",      # pool | dve | mix
    "at_eng": "act",       # act | dve
    "vh_eng": "act",       # act | dve
    "ptrow": "act",        # mix | act | dve
}

# power-of-two scale exponents (see module docstring)
#   proj psum: 2^17 = (w_hi*2^8)(x_hi*2^9); fp8 planes (w*2^6)(xres*2^11),
#              (wres*2^16)(x*2^1)
#   scores:    2^15 = (qh_hi*2^7)(kh_hi*2^8); fp8 (qres*2^13)(khi*2^2),
#              (qhi*2^2)(kres*2^13)
E_WH, E_XH = 8, 9
E_W8, E_XR8 = 6, 11
E_WR8, E_XF8 = 16, 1
E_QH, E_KH = 7, 8  # qhT_hi/khT_hi sbuf scales
SC_SCORE = 2.0 ** -15


def _f8(a):
    return np.clip(np.asarray(a, np.float32), -240, 240).astype(NPF8)


def _build_program(k_index: int):
    """Builds the per-core Bass program."""
    nc = bacc.Bacc(
        "TRN2", target_bir_lowering=False, debug=False, num_devices=NCORES
    )

    # --- DRAM I/O -------------------------------------------------------
    qTh = nc.dram_tensor("qTh", (BC, D, S), F16, kind="ExternalInput").ap()
    kTh = nc.dram_tensor("kTh", (BC, D, S), F16, kind="ExternalInput").ap()
    qc8 = nc.dram_tensor("qc8", (BC, FT, 128, 2, S), F8, kind="ExternalInput").ap()
    kc8 = nc.dram_tensor("kc8", (BC, FT, 128, 2, S), F8, kind="ExternalInput").ap()
    vT = nc.dram_tensor("vT", (BC, D, S), F16, kind="ExternalInput").ap()
    wqh = nc.dram_tensor("wqh", (D, D), F16, kind="ExternalInput").ap()
    wkh = nc.dram_tensor("wkh", (D, D), F16, kind="ExternalInput").ap()
    wqc8 = nc.dram_tensor("wqc8", (FT, 128, 2, D), F8, kind="ExternalInput").ap()
    wkc8 = nc.dram_tensor("wkc8", (FT, 128, 2, D), F8, kind="ExternalInput").ap()
    wv = nc.dram_tensor("wv", (D, D), F16, kind="ExternalInput").ap()
    wo = nc.dram_tensor("wo", (D, D), F16, kind="ExternalInput").ap()
    out = nc.dram_tensor("out", (BC, S, D), F32, kind="ExternalOutput").ap()

    # --- inline constants ----------------------------------------------
    ident_np = np.eye(128, dtype=np.float32)
    mask_np = np.where(
        np.arange(128)[None, :] >= np.arange(128)[:, None], NEG, 0.0
    ).astype(np.float32)
    ident_p = nc.inline_tensor(
        ident_np.astype(np.float16), name="identp"
    ).ap()
    ident_b = nc.inline_tensor(
        ident_np.astype(mybir.dt.np(BF16)), name="identb"
    ).ap()
    maskT_b = nc.inline_tensor(
        mask_np.T.copy().astype(mybir.dt.np(BF16)), name="maskT"
    ).ap()

    DR = mybir.MatmulPerfMode.DoubleRow
    Exp = mybir.ActivationFunctionType.Exp
    Copy = mybir.ActivationFunctionType.Copy
    AO = mybir.AluOpType

    with tile.TileContext(nc) as tc, ExitStack() as ctx:
        # ---------------- pools ----------------
        consts = ctx.enter_context(tc.tile_pool(name="consts", bufs=1))
        xpool = ctx.enter_context(tc.tile_pool(name="xpool", bufs=CFG["xpool"]))
        projpool = ctx.enter_context(
            tc.tile_pool(name="projpool", bufs=CFG["projpool"]))
        lopool = ctx.enter_context(tc.tile_pool(name="lopool", bufs=2))
        epool = ctx.enter_context(tc.tile_pool(name="epool", bufs=CFG["epool"]))
        ppool = ctx.enter_context(tc.tile_pool(name="ppool", bufs=CFG["ppool"]))
        pnpool = ctx.enter_context(tc.tile_pool(name="pnpool", bufs=CFG["pnpool"]))
        ptpool = ctx.enter_context(tc.tile_pool(name="ptpool", bufs=CFG["ptpool"]))
        smallpool = ctx.enter_context(tc.tile_pool(name="smallpool", bufs=4))
        atpool = ctx.enter_context(tc.tile_pool(name="atpool", bufs=3))
        ypool = ctx.enter_context(tc.tile_pool(name="ypool", bufs=3))

        ps_proj = ctx.enter_context(
            tc.tile_pool(name="ps_proj", bufs=CFG["ps_proj"], space="PSUM"))
        ps_sc = ctx.enter_context(
            tc.tile_pool(name="ps_sc", bufs=CFG["ps_sc"], space="PSUM"))
        ps_pt = ctx.enter_context(
            tc.tile_pool(name="ps_pt", bufs=CFG["ps_pt"], space="PSUM"))
        ps_at = ctx.enter_context(
            tc.tile_pool(name="ps_at", bufs=CFG["ps_at"], space="PSUM"))
        ps_y = ctx.enter_context(
            tc.tile_pool(name="ps_y", bufs=CFG["ps_y"], space="PSUM"))

        # ---------------- resident constants ----------------
        # One wide DMA per tensor (the SP sequencer serializes DMA issue at
        # ~565ns each, so fewer/bigger transfers matter): dram [(f p) s]
        # lands as a [128, f*s] tile whose column block f holds partition
        # rows f*128..f*128+127.
        def load_blocked(pool, ap2d, name):
            t = pool.tile_from(
                ap2d.rearrange("(f p) s -> p f s", p=128), name=name)
            return [t[:, ft] for ft in range(FT)]

        def load_c8(pool, ap4d, name):
            # dram [f p two s] -> tile [128, f, 2, s]; per-ft [128, 2, s] views
            t = pool.tile_from(
                ap4d.rearrange("f p two s -> p f two s"), name=name)
            return [t[:, ft] for ft in range(FT)]

        # q weights first, then batch 0's q activations, so the first
        # projection matmuls start as early as possible on the DMA queue.
        wqh_sb = load_blocked(consts, wqh, "wqh")
        wqc8_sb = load_c8(consts, wqc8, "wqc8")
        _xq0h = load_blocked(xpool, qTh[0], "xqh")
        _xq0c = load_c8(xpool, qc8[0], "xqc")
        wkh_sb = load_blocked(consts, wkh, "wkh")
        wkc8_sb = load_c8(consts, wkc8, "wkc8")
        preloaded = {}
        preloaded[0] = (
            _xq0h, _xq0c,
            load_blocked(xpool, kTh[0], "xkh"),
            load_c8(xpool, kc8[0], "xkc"),
            load_blocked(xpool, vT[0], "xv"),
        )
        wv_sb = load_blocked(consts, wv, "wv")
        wo_sb = load_blocked(consts, wo, "wo")
        identp_sb = consts.tile_from(ident_p, name="identp_sb")
        identb_sb = consts.tile_from(ident_b, name="identb_sb")
        maskT_sb = consts.tile_from(maskT_b, name="maskT_sb")

        def emit_proj(b, defer_v=False):
            """Loads + q/k/v projections for batch b.

            q/k: per head-pair dt, one 2^17-scaled PSUM accumulates 4 fp16
            main matmuls + 4 fp8 DoubleRow correction matmuls; evacuated as
            a 2^E_QH/E_KH-scaled fp16 hi tile, an fp16 lo residual, and a
            stacked fp8 [lo|hi] (q) / [hi|lo] (k) pair tile for the QK
            cross-term DoubleRow matmuls.
            """
            if b in preloaded:
                xqh, xqc, xkh, xkc, xv = preloaded.pop(b)
            else:
                xqh = load_blocked(xpool, qTh[b], "xqh")
                xqc = load_c8(xpool, qc8[b], "xqc")
                xkh = load_blocked(xpool, kTh[b], "xkh")
                xkc = load_c8(xpool, kc8[b], "xkc")
                xv = load_blocked(xpool, vT[b], "xv")
            qhT, khT, qc8t, kc8t, vh = [], [], [], [], []
            for dt in range(FT):
                for which, wh_sb, wc_sb, xh, xc in (
                        ("q", wqh_sb, wqc8_sb, xqh, xqc),
                        ("k", wkh_sb, wkc8_sb, xkh, xkc)):
                    ps = ps_proj.tile([128, S], F32, name="psq", tag="psproj")
                    for ft in range(FT):
                        nc.tensor.matmul(
                            ps, wh_sb[ft][:, dt * 128:(dt + 1) * 128], xh[ft],
                            start=(ft == 0), stop=False)
                    for ft in range(FT):
                        nc.tensor.matmul(
                            ps, wc_sb[ft][:, :, dt * 128:(dt + 1) * 128], xc[ft],
                            start=False, stop=(ft == FT - 1), perf_mode=DR)
                    hi = projpool.tile([128, S], F16, name=f"{which}hT{dt}",
                                       tag=f"{which}hT{dt}")
                    lo = lopool.tile([128, S], F16, name=f"{which}lo",
                                     tag=f"{which}lo{dt % 2}")
                    c8 = projpool.tile([128, 2 * S], F8, name=f"{which}c8{dt}",
                                       tag=f"{which}c8{dt}")
                    eh = E_QH if which == "q" else E_KH
                    # psum 2^17 -> hi 2^eh (ACT), lo residual (DVE; GPSIMD
                    # cannot read PSUM), fp8 planes (DVE, f16->f8)
                    nc.scalar.activation(hi, ps, Copy, scale=2.0 ** (eh - 17))
                    nc.vector.scalar_tensor_tensor(
                        lo, ps, 2.0 ** (eh - 17), hi,
                        op0=AO.mult, op1=AO.subtract)
                    if which == "q":
                        # plane0 = qres*2^13 (Pool), plane1 = qhi*2^2 (DVE)
                        nc.gpsimd.tensor_scalar(
                            c8[:, 0:S], lo, 2.0 ** (13 - eh), None, op0=AO.mult)
                        nc.vector.tensor_scalar(
                            c8[:, S:2 * S], hi, 2.0 ** (2 - eh), None, op0=AO.mult)
                        qhT.append(hi)
                        qc8t.append(c8)
                    else:
                        # plane0 = khi*2^2 (DVE), plane1 = kres*2^13 (Pool)
                        nc.vector.tensor_scalar(
                            c8[:, 0:S], hi, 2.0 ** (2 - eh), None, op0=AO.mult)
                        nc.gpsimd.tensor_scalar(
                            c8[:, S:2 * S], lo, 2.0 ** (13 - eh), None, op0=AO.mult)
                        khT.append(hi)
                        kc8t.append(c8)

            def do_vproj(rts=range(RT)):
                for rt in rts:
                    ps = ps_proj.tile([128, D], F32, name="psv", tag="psproj")
                    for ft in range(FT):
                        nc.tensor.matmul(
                            ps, xv[ft][:, rt * 128:(rt + 1) * 128], wv_sb[ft],
                            start=(ft == 0), stop=(ft == FT - 1))
                    t = projpool.tile([128, D], F16, name=f"vh{rt}", tag=f"vh{rt}")
                    if CFG["vh_eng"] == "act":
                        nc.scalar.copy(t, ps)
                    else:
                        nc.vector.tensor_copy(t, ps)
                    vh.append(t)
                return vh
            if defer_v:
                return qhT, khT, qc8t, kc8t, do_vproj
            return qhT, khT, qc8t, kc8t, do_vproj()

        def emit_headpair(hp, qhT, khT, qc8t, kc8t, vh):
            """Scores / top-k softmax / transposes / attnT for one head pair."""
            qc8v = qc8t[hp].rearrange("p (two s) -> p two s", two=2)
            kc8v = kc8t[hp].rearrange("p (two s) -> p two s", two=2)
            etiles = [[None] * RT, [None] * RT]
            top8s = []
            for hh in range(2):
                top8s.append(smallpool.tile(
                    [128, RT * 8], F32, name=f"top8{hh}", tag=f"top8{hh}"))
            for ri in range(RT):
                w = (ri + 1) * 128
                spss = []
                for hh in range(2):
                    po = hh * 64
                    sps = ps_sc.tile([128, S], F32, name="sps", tag="sps")
                    nc.tensor.matmul(
                        sps[:, 0:w],
                        qhT[hp][po:po + 64, ri * 128:(ri + 1) * 128],
                        khT[hp][po:po + 64, 0:w],
                        start=True, stop=False)
                    spss.append(sps)
                for hh in range(2):
                    po = hh * 64
                    nc.tensor.matmul(
                        spss[hh][:, 0:w],
                        qc8v[po:po + 64, :, ri * 128:(ri + 1) * 128],
                        kc8v[po:po + 64, :, 0:w],
                        start=False, stop=False, perf_mode=DR)
                for hh in range(2):
                    nc.tensor.matmul(
                        spss[hh][:, ri * 128:(ri + 1) * 128],
                        maskT_sb, identb_sb, start=False, stop=True)
                for hh in range(2):
                    e = epool.tile([128, S], F32, name="e", tag="e")
                    nc.scalar.activation(
                        e[:, 0:w], spss[hh][:, 0:w], Exp, scale=SC_SCORE)
                    nc.vector.max(
                        out=top8s[hh][:, ri * 8:(ri + 1) * 8], in_=e[:, 0:w])
                    etiles[hh][ri] = e
            ptrows = [[None] * RT, [None] * RT]
            pns2 = [[], []]
            for hh in range(2):
                # Rows with fewer than k_index valid (strictly-causal) entries
                # naturally have top8[k-1] == 0, so tau == 0 keeps everything
                # and sum(top-k) equals the full row sum — no special-casing
                # needed beyond row 0 (all-zero row: Z := 1 to avoid 1/0).
                top8 = top8s[hh]
                zk = smallpool.tile([128, RT], F32, name="zk", tag="zk")
                nc.vector.reduce_sum(
                    zk,
                    top8.rearrange("p (r e) -> p r e", e=8)[:, :, 0:k_index],
                    axis=mybir.AxisListType.X)
                nc.vector.memset(zk[0:1, 0:1], 1.0)
                rz = smallpool.tile([128, RT], F32, name="rz", tag="rz")
                nc.vector.reciprocal(rz, zk)

                for ri in range(RT):
                    w = (ri + 1) * 128
                    e = etiles[hh][ri]
                    tau = top8[:, ri * 8 + k_index - 1: ri * 8 + k_index]
                    pu = ppool.tile([128, S], F16, name="pu", tag="pu")
                    nc.vector.scalar_tensor_tensor(
                        pu[:, 0:w], e[:, 0:w], tau, e[:, 0:w],
                        op0=AO.is_ge, op1=AO.mult)
                    pn = pnpool.tile([128, S], F16, name="pn", tag="pn")
                    pn_eng = CFG["pn_eng"]
                    use_dve = pn_eng == "dve" or (pn_eng == "mix" and ri % 2)
                    (nc.vector if use_dve else nc.gpsimd).tensor_scalar(
                        pn[:, 0:w], pu[:, 0:w], rz[:, ri:ri + 1], None,
                        op0=AO.mult)
                    pns2[hh].append(pn)
            for hh in range(2):
                for ci in range(RT):
                    wv_ = (RT - ci) * 128
                    ptb = ps_pt.tile([128, S], F16, name="ptb", tag="ptb")
                    for ri in range(ci, RT):
                        nc.tensor.transpose(
                            ptb[:, (ri - ci) * 128:(ri - ci + 1) * 128],
                            pns2[hh][ri][:, ci * 128:(ci + 1) * 128],
                            identp_sb)
                    ptrow = ptpool.tile([128, S], F16, name="ptrow",
                                        tag="ptrow")
                    pm = CFG["ptrow"]
                    use_dve = pm == "dve" or (pm == "mix" and ci % 2 == 0)
                    if use_dve:
                        nc.vector.tensor_copy(ptrow[:, 0:wv_], ptb[:, 0:wv_])
                    else:
                        nc.scalar.copy(ptrow[:, 0:wv_], ptb[:, 0:wv_])
                    ptrows[hh][ci] = ptrow[:, 0:wv_]

            def finish(vh):
                at_ps = ps_at.tile([128, S], F32, name="atps", tag="atps")
                for ci in range(RT):
                    wv_ = (RT - ci) * 128
                    for hh in range(2):
                        h = 2 * hp + hh
                        nc.tensor.matmul(
                            at_ps[hh * 64:hh * 64 + 64, ci * 128:S],
                            vh[ci][:, h * DK:(h + 1) * DK],
                            ptrows[hh][ci][:, 0:wv_],
                            start=(ci == 0), stop=(ci == RT - 1),
                            skip_group_check=True)
                at = atpool.tile([128, S], F16, name=f"at{hp}", tag=f"at{hp}")
                if CFG["at_eng"] == "act":
                    nc.scalar.copy(at, at_ps)
                else:
                    nc.vector.tensor_copy(at, at_ps)
                return at
            if vh is None:
                return finish
            return finish(vh)

        def emit_y(b, attnT_sb):
            for ri in range(RT):
                yps = ps_y.tile([128, D], F32, name="yps", tag="yps")
                for hp in range(FT):
                    nc.tensor.matmul(
                        yps, attnT_sb[hp][:, ri * 128:(ri + 1) * 128], wo_sb[hp],
                        start=(hp == 0), stop=(hp == FT - 1))
                y = ypool.tile([128, D], F32, name="y", tag="y")
                nc.scalar.copy(y, yps)
                nc.scalar.dma_start(out[b, ri * 128:(ri + 1) * 128, :], y)

        for b in range(BC):
            last = b == BC - 1
            qhT, khT, qc8t, kc8t, vh = emit_proj(b, defer_v=last)
            attnT_sb = []
            if last:
                # cooldown filler: spread the last batch's v-projection
                # groups across the head-pair phases
                do_v = vh
                fins = []
                vh = None
                for hp in range(FT):
                    fins.append(emit_headpair(hp, qhT, khT, qc8t, kc8t, None))
                    vh = do_v(rts=[hp])
                attnT_sb = [fin(vh) for fin in fins]
            else:
                for hp in range(FT):
                    attnT_sb.append(
                        emit_headpair(hp, qhT, khT, qc8t, kc8t, vh))
            emit_y(b, attnT_sb)

    nc.compile()
    return nc


def _prep_side(x, w):
    """Host split of one projection input pair.

    x: [n, S, D] fp32 activations, w: [D, D] fp32 weights (score scale
    pre-folded for q).  Returns (xTh fp16 [n,D,S], xc8 fp8 [n,FT,128,2,S],
    wh fp16 [D,D], wc8 fp8 [FT,128,2,D]).
    """
    x = np.asarray(x, np.float64)
    w = np.asarray(w, np.float64)
    x_hi = x.astype(np.float32).astype(np.float16)
    x_res = x - x_hi.astype(np.float64)
    w_hi = w.astype(np.float32).astype(np.float16)
    w_res = w - w_hi.astype(np.float64)

    xTh = np.ascontiguousarray(
        (x_hi.astype(np.float32) * 2.0 ** E_XH).astype(np.float16)
        .transpose(0, 2, 1))
    n = x.shape[0]
    xc8 = np.empty((n, FT, 128, 2, S), NPF8)
    p0 = _f8(x_res * 2.0 ** E_XR8).transpose(0, 2, 1).reshape(n, FT, 128, S)
    p1 = _f8(x * 2.0 ** E_XF8).transpose(0, 2, 1).reshape(n, FT, 128, S)
    xc8[:, :, :, 0, :] = p0
    xc8[:, :, :, 1, :] = p1

    wh = np.ascontiguousarray(
        (w_hi.astype(np.float32) * 2.0 ** E_WH).astype(np.float16))
    wc8 = np.empty((FT, 128, 2, D), NPF8)
    wc8[:, :, 0, :] = _f8(w * 2.0 ** E_W8).reshape(FT, 128, D)
    wc8[:, :, 1, :] = _f8(w_res * 2.0 ** E_WR8).reshape(FT, 128, D)
    return xTh, np.ascontiguousarray(xc8), wh, np.ascontiguousarray(wc8)


def kernel(**inputs):
    q = np.asarray(inputs["q"], np.float32)
    k = np.asarray(inputs["k"], np.float32)
    v = np.asarray(inputs["v"], np.float32)
    w_q = np.asarray(inputs["w_q"], np.float32)
    w_k = np.asarray(inputs["w_k"], np.float32)
    w_v = np.asarray(inputs["w_v"], np.float32)
    w_o = np.asarray(inputs["w_o"], np.float32)
    b_q = np.asarray(inputs["b_q"], np.float32)
    b_k = np.asarray(inputs["b_k"], np.float32)
    b_v = np.asarray(inputs["b_v"], np.float32)
    b_o = np.asarray(inputs["b_o"], np.float32)
    k_index = int(np.asarray(inputs["k_index"]))
    assert 1 <= k_index <= 8, f"kernel supports k_index<=8, got {k_index}"
    assert not (np.any(b_q) or np.any(b_k) or np.any(b_v) or np.any(b_o)), (
        "this kernel build assumes zero biases")

    # fold the 1/sqrt(DK) score scaling into the q projection
    w_qs = (w_q.astype(np.float64) / math.sqrt(DK))

    nc = _build_program(k_index)
    global _last_nc
    _last_nc = nc

    _, _, wqh, wqc8 = _prep_side(q[:1], w_qs)
    _, _, wkh, wkc8 = _prep_side(k[:1], w_k)
    shared = {
        "wqh": wqh, "wqc8": wqc8, "wkh": wkh, "wkc8": wkc8,
        "wv": np.ascontiguousarray(w_v.astype(np.float16)),
        "wo": np.ascontiguousarray(w_o.astype(np.float16)),
    }

    in_maps = []
    for c in range(NCORES):
        sl = slice(c * BC, (c + 1) * BC)
        qTh, qc8_, _, _ = _prep_side(q[sl], w_qs)
        kTh, kc8_, _, _ = _prep_side(k[sl], w_k)
        in_maps.append(dict(
            shared,
            qTh=qTh, qc8=qc8_, kTh=kTh, kc8=kc8_,
            vT=np.ascontiguousarray(
                v[sl].transpose(0, 2, 1).astype(np.float16)),
        ))

    res = run_bass_kernel_spmd(
        nc, in_maps, core_ids=list(range(NCORES)), trace=CFG["trace"]
    )
    out = np.concatenate([r["out"] for r in res.results], axis=0)
    kernel.last_result = res
    return out
